# revision 1
# baseline (speedup 1.0000x reference)
"""Trainium2 Bass kernel for nn_MoE_29927332118881.

Math (reference): for each of N=48 rows (B=16 x CH=3) and K=4 anisotropic
2-D Gaussians over a 384x384 unit grid:
    e_k(h,w) = exp(-0.5 * || Sigma_k^T d ||^2),  d = (x(h)-mux_k, y(w)-muy_k)
    out = clip( sum_k w_k e_k / max(eps, sum_k e_k), 0, 1 )

Factorization used on device (per row n, kernel k):
    u(h,w)   = qa_k(h) + dxB_k(h) * (y(w) - muy_k) + qc_k(w)
    e_k      = exp(qa_k(h) + dxB_k(h)*y(w) + bias0_k(h)) * c_k(w)
with qa, dxB, bias0 per-partition (h) scalars consumed by ScalarE's free
affine (out = exp(scale*in + bias)), in = broadcast y-grid tile, and
c_k(w) = exp(qc_k(w)) a per-(k,w) tile multiplied in on VectorE.

Layout: partitions pack (k in 4) x (j in 32 h-rows); a 384-row output chunk
is 4 blocks of 32 h-rows x 128 partitions. The k-contraction (sum_k w_k e
and sum_k e) runs on TensorE as [128,32] stationaries into col-tiled PSUM.

Sharding: data-parallel over the params batch dim: core i gets batch rows
[2i, 2i+2) = 6 of the 48 (B,CH) rows. The grid is replicated.
"""

import os
import numpy as np

import concourse.bass as bass
import concourse.bacc as bacc
import concourse.mybir as mybir
from concourse.tile import TileContext
from concourse.bass_utils import run_bass_kernel_spmd

import concourse.dve_ops as dve_ops_mod
from concourse.dve_spec import Spec, Src0, Src1, C0, relu, minn, lower, _has_src1
from concourse.dve_uop import DveOpSpec


def _ensure_clip_mul_op():
    """Register a fused custom-DVE op: out = min(relu(in0) * in1, s0).
    One Vector instruction for the whole finalize tail (y*r, relu, clamp)."""
    for op in dve_ops_mod.OPS:
        if op.name == "CLIP_MUL_ANT":
            return op
    spec = Spec(
        body=minn(relu(Src0) * Src1, C0),
        reference=lambda in0, in1, s0, s1, imm2: np.minimum(
            np.maximum(in0.astype(np.float32), 0.0) * in1, s0
        ).astype(np.float32),
    )
    row = max(dve_ops_mod._SUB_OPCODE_FOR_NAME.values()) + 1
    assert row < 0x20
    dve_ops_mod._SUB_OPCODE_FOR_NAME["CLIP_MUL_ANT"] = row
    shas = {}
    for ver in ("v3", "v4"):
        s = DveOpSpec(name="CLIP_MUL_ANT", opcode=row,
                      uops=lower(spec, ver=ver), rd1_en=_has_src1(spec))
        shas[ver] = s.sha(ver)
    op = dve_ops_mod.DveOp("CLIP_MUL_ANT", spec, False, shas)
    dve_ops_mod.OPS.append(op)
    dve_ops_mod.CUSTOM_DVE_SPECS["CLIP_MUL_ANT"] = spec
    return op

F32 = mybir.dt.float32
BF16 = mybir.dt.bfloat16

H = 384          # x / partition-chunk axis (first grid coordinate)
W = 384          # y / free axis (second grid coordinate)
K = 4
N_CORES = 8
N_LOC = 6        # (B*CH) rows per core
BLK = 32         # h-rows per block; 4 blocks per 128-partition chunk
N_BLOCKS = H // BLK          # 12
N_CHUNKS = H // 128          # 3
EPS = 1e-7

# Product-path dtype mode:
#   "f16": exp output, c factor, product, and matmul stationaries all fp16
#          (DVE tensor_tensor at 2x rate, best accuracy of the 16-bit modes;
#          requires exp(u1) < 60000, guarded at runtime from the params)
#   "f32": exp output and c fp32 (DVE 1x), product cast to bf16 for TensorE
PROD_DT = os.environ.get("KERNEL_PROD_DT", "f16")
F16 = mybir.dt.float16

_cache = {}


def _build_nc(prod_dt):
    nc = bacc.Bacc(target_bir_lowering=False)

    if prod_dt == "f16":
        cdt = pdt = sdt = F16
    else:
        cdt = F32       # exp output / c factor dtype
        pdt = BF16      # product (matmul moving) dtype
        sdt = BF16      # stationary dtype

    NB = N_LOC * N_BLOCKS
    # consts: [:, :W] = y-grid bcast; [:, W:W+NB] = scale; [:, W+NB:] = bias
    consts_d = nc.dram_tensor("consts", [128, W + 2 * NB], F32,
                              kind="ExternalInput")
    c_d = nc.dram_tensor("c_pack", [N_LOC, 128, W], cdt, kind="ExternalInput")
    # stat: [:, :BLK] = sg identity; [:, BLK + n*BLK :] = sy for row n
    stat_d = nc.dram_tensor("stat", [128, (N_LOC + 1) * BLK], sdt,
                            kind="ExternalInput")
    out_d = nc.dram_tensor("out", [N_LOC, H, W], F32, kind="ExternalOutput")

    clip_op = _ensure_clip_mul_op()

    with TileContext(nc) as tc:
        with (
            tc.tile_pool(name="const", bufs=1) as constp,
            # one slot per block: no buffer reuse -> no WAR waits on the
            # scalar/vector queues (each wait costs ~100ns of queue time);
            # the fp32 fallback path halves the depth to fit SBUF
            tc.tile_pool(name="epp",
                         bufs=N_LOC * N_CHUNKS if prod_dt == "f16" else 6) as epp,
            tc.tile_pool(name="ebfp",
                         bufs=N_LOC * N_CHUNKS if prod_dt == "f16" else 6) as ebfp,
            tc.tile_pool(name="work", bufs=18) as work,
            tc.tile_pool(name="fin", bufs=8) as fin,
            tc.tile_pool(name="ypsum", bufs=4, space="PSUM") as ypsum,
            tc.tile_pool(name="gpsum", bufs=4, space="PSUM") as gpsum,
        ):
            consts = constp.tile([128, W + 2 * NB], F32)
            nc.gpsimd.dma_start(out=consts[:], in_=consts_d[:])
            ybc = consts[:, 0:W]
            scl = consts[:, W:W + NB]
            bia = consts[:, W + NB:W + 2 * NB]
            stat = constp.tile([128, (N_LOC + 1) * BLK], sdt)
            nc.gpsimd.dma_start(out=stat[:], in_=stat_d[:])
            sgt = stat[:, 0:BLK]
            c_all = constp.tile([128, N_LOC, W], cdt)
            for n in range(N_LOC):
                nc.gpsimd.dma_start(out=c_all[:, n, :], in_=c_d[n])

            for n in range(N_LOC):
                for c in range(N_CHUNKS):
                    yp = ypsum.tile([128, W], F32)
                    gp = gpsum.tile([128, W], F32)
                    ep = epp.tile([128, 4, W], cdt)
                    ebf = ebfp.tile([128, 4, W], pdt)
                    for s in range(4):
                        b = c * 4 + s
                        col = n * N_BLOCKS + b
                        nc.scalar.activation(
                            ep[:, s, :], ybc, mybir.ActivationFunctionType.Exp,
                            bias=bia[:, col:col + 1],
                            scale=scl[:, col:col + 1],
                        )
                        nc.vector.tensor_tensor(
                            out=ebf[:, s, :], in0=ep[:, s, :],
                            in1=c_all[:, n, :],
                            op=mybir.AluOpType.mult,
                        )
                        nc.tensor.matmul(
                            yp[BLK * s: BLK * (s + 1), :],
                            stat[:, (1 + n) * BLK:(2 + n) * BLK], ebf[:, s, :],
                            start=True, stop=True,
                            tile_position=(0, BLK * s),
                        )
                        nc.tensor.matmul(
                            gp[BLK * s: BLK * (s + 1), :],
                            sgt, ebf[:, s, :],
                            start=True, stop=True,
                            tile_position=(0, BLK * s),
                        )
                    # rebalance: DVE is the tail engine, ACT has ~8% slack;
                    # every ACT_RECIP_MOD-th chunk computes 1/g on ScalarE as
                    # exp(-ln(g)) (both funcs live in one ACT table set)
                    chunk_idx = n * N_CHUNKS + c
                    act_recip_mod = int(os.environ.get("KERNEL_ACT_RECIP_MOD", "0"))
                    r = work.tile([128, W], F32)
                    if act_recip_mod and chunk_idx % act_recip_mod == 0:
                        lng = work.tile([128, W], F32)
                        nc.scalar.activation(
                            lng[:], gp[:], mybir.ActivationFunctionType.Ln)
                        nc.scalar.activation(
                            r[:], lng[:], mybir.ActivationFunctionType.Exp,
                            scale=-1.0)
                    elif os.environ.get("KERNEL_FAST_RECIP", "1") == "1":
                        nc.vector.reciprocal_approx_fast(out=r[:], in_=gp[:])
                    else:
                        nc.vector.reciprocal(out=r[:], in_=gp[:])
                    oc = fin.tile([128, W], F32)
                    # oc = min(relu(yp) * r, 1)  (r > 0: == clip(yp*r, 0, 1))
                    nc.vector._custom_dve(
                        clip_op, out=oc[:], in0=yp[:], in1=r[:], s0=1.0,
                    )
                    nc.sync.dma_start(
                        out=out_d[n, 128 * c: 128 * (c + 1), :], in_=oc[:],
                    )
    nc.finalize()
    return nc


def _host_precompute(params: np.ndarray, prod_dt: str):
    """Build the per-core derived input arrays (float64 host math)."""
    P = np.asarray(params, dtype=np.float64).reshape(48, 28)
    mu_x = P[:, 0:4]
    mu_y = P[:, 4:8]
    wgt = P[:, 8:12]
    S00 = P[:, 12:28][:, 0::4]
    S10 = P[:, 12:28][:, 2::4]
    S11 = P[:, 12:28][:, 3::4]
    A = S00 ** 2
    Bq = 2.0 * S00 * S10
    C = S10 ** 2 + S11 ** 2

    x = np.linspace(0.0, 1.0, H)
    y = np.linspace(0.0, 1.0, W)

    import ml_dtypes
    bf = ml_dtypes.bfloat16

    NB = N_LOC * N_BLOCKS
    cdt = np.float16 if prod_dt == "f16" else np.float32
    sdt = np.float16 if prod_dt == "f16" else bf

    in_maps = []
    for core in range(N_CORES):
        consts = np.zeros((128, W + 2 * NB), dtype=np.float64)
        consts[:, 0:W] = y[None, :]
        stat = np.zeros((128, (N_LOC + 1) * BLK), dtype=np.float32)
        for k in range(K):
            stat[k * BLK:(k + 1) * BLK, 0:BLK] = np.eye(BLK)
        c_pack = np.zeros((N_LOC, 128, W), dtype=np.float64)
        for n in range(N_LOC):
            ng = core * N_LOC + n
            for k in range(K):
                dx = x - mu_x[ng, k]
                dxB = -0.5 * Bq[ng, k] * dx
                qa = -0.5 * A[ng, k] * dx * dx
                rows = slice(k * BLK, (k + 1) * BLK)
                # col = n*12 + b holds h-rows [32b, 32b+32)
                cols = slice(W + n * N_BLOCKS, W + (n + 1) * N_BLOCKS)
                consts[rows, cols] = dxB.reshape(N_BLOCKS, BLK).T
                cols = slice(W + NB + n * N_BLOCKS, W + NB + (n + 1) * N_BLOCKS)
                consts[rows, cols] = (qa - dxB * mu_y[ng, k]).reshape(N_BLOCKS, BLK).T
                dy = y - mu_y[ng, k]
                c_pack[n, rows, :] = np.exp(-0.5 * C[ng, k] * dy * dy)[None, :]
                stat[rows, (1 + n) * BLK:(2 + n) * BLK] = \
                    np.eye(BLK) * wgt[ng, k]
        in_maps.append({
            "consts": consts.astype(np.float32),
            "c_pack": c_pack.astype(np.float32).astype(cdt),
            "stat": stat.astype(sdt),
        })
    return in_maps


def _u1_max(params: np.ndarray) -> float:
    """Exact max over the grid of the exp argument (qa + dxB*(y-muy));
    used to guard the fp16 product path against overflow."""
    P = np.asarray(params, dtype=np.float64).reshape(48, 28)
    mu_x, mu_y = P[:, 0:4], P[:, 4:8]
    S00 = P[:, 12:28][:, 0::4]
    S10 = P[:, 12:28][:, 2::4]
    A, Bq = S00 ** 2, 2.0 * S00 * S10
    x = np.linspace(0.0, 1.0, H)
    y = np.linspace(0.0, 1.0, W)
    m = -np.inf
    for ng in range(48):
        for k in range(K):
            dx = x - mu_x[ng, k]
            u1 = (-0.5 * A[ng, k] * dx * dx)[:, None] \
                + (-0.5 * Bq[ng, k] * dx)[:, None] * (y - mu_y[ng, k])[None, :]
            m = max(m, float(u1.max()))
    return m


def _run(height, width, params, trace=False, **trace_kwargs):
    assert int(height) == H and int(width) == W, (height, width)
    prod_dt = PROD_DT
    if prod_dt == "f16" and _u1_max(params) > 10.5:
        prod_dt = "f32"   # exp would overflow fp16; use the fp32 path
    if prod_dt not in _cache:
        _cache[prod_dt] = _build_nc(prod_dt)
    nc = _cache[prod_dt]
    in_maps = _host_precompute(params, prod_dt)
    res = run_bass_kernel_spmd(
        nc, in_maps, core_ids=list(range(N_CORES)), trace=trace, **trace_kwargs
    )
    full = np.empty((48, H, W), dtype=np.float32)
    for core in range(N_CORES):
        full[core * N_LOC:(core + 1) * N_LOC] = res.results[core]["out"]
    return full.reshape(16, 3, H, W), res


def kernel(height, width, params):
    out, _ = _run(height, width, params)
    return out



# revision 11
# speedup vs baseline: 1.9702x; 1.9702x over previous
"""Trainium2 Bass kernel for nn_MoE_29927332118881.

Math (reference): for each of N=48 rows (B=16 x CH=3) and K=4 anisotropic
2-D Gaussians over a 384x384 unit grid:
    e_k(h,w) = exp(-0.5 * || Sigma_k^T d ||^2),  d = (x(h)-mux_k, y(w)-muy_k)
    out = clip( sum_k w_k e_k / max(eps, sum_k e_k), 0, 1 )

Strategy (validated numerically on host, absmax err ~4e-3 vs 2e-2 tol):
the Gaussians are very smooth on this grid, so evaluate exp only on a
coarse h-grid (8 samples per 128-row output chunk), contract k and
cubic-interpolate to a 16-row mid grid in one TensorE matmul, take the
normalized ratio y = clip(num/den,0,1) on the mid grid (VectorE recip +
fused clip-mul), cubic-interpolate mid->fine on TensorE (interp weights
baked into f16 stationaries), then evict PSUM->SBUF as f16 (split
between ScalarE and VectorE -- the only PSUM-capable engines) and DMA.

The separable exp factor exp(qc(w)) is folded into the ACT input tile:
exp(scale*(y + qc/scale) + bias) with a |scale|<tau fallback (input=qc,
scale=1, error < tau) -- no elementwise multiply needed anywhere.

Layout: chunk t (n = t//3 of the 6 local rows, c = t%3 of 3 h-chunks)
owns partitions 32q:32q+32 = (k=4 x j=8 coarse rows) of ACT group
g = t//4 (q = t%4). Supers of 2 groups (8 chunks) share one [128,384]
num/den PSUM pair. 18 real chunks are padded with 2 dummy chunks
(e=1, num-weights=0) so every PSUM row read is written.

Sharding: data-parallel over the params batch dim: core i gets 6 of the
48 (B,CH) rows. The grid is replicated.
"""

import numpy as np

import concourse.bass as bass
import concourse.bacc as bacc
import concourse.mybir as mybir
from concourse.tile import TileContext
from concourse.bass_utils import run_bass_kernel_spmd

import concourse.dve_ops as dve_ops_mod
from concourse.dve_spec import Spec, Src0, Src1, C0, relu, minn, lower, _has_src1
from concourse.dve_uop import DveOpSpec


def _ensure_clip_mul_op():
    """Fused custom-DVE op: out = min(relu(in0) * in1, s0)."""
    for op in dve_ops_mod.OPS:
        if op.name == "CLIP_MUL_ANT":
            return op
    spec = Spec(
        body=minn(relu(Src0) * Src1, C0),
        reference=lambda in0, in1, s0, s1, imm2: np.minimum(
            np.maximum(in0.astype(np.float32), 0.0) * in1, s0
        ).astype(np.float32),
    )
    row = max(dve_ops_mod._SUB_OPCODE_FOR_NAME.values()) + 1
    assert row < 0x20
    dve_ops_mod._SUB_OPCODE_FOR_NAME["CLIP_MUL_ANT"] = row
    shas = {}
    for ver in ("v3", "v4"):
        s = DveOpSpec(name="CLIP_MUL_ANT", opcode=row,
                      uops=lower(spec, ver=ver), rd1_en=_has_src1(spec))
        shas[ver] = s.sha(ver)
    op = dve_ops_mod.DveOp("CLIP_MUL_ANT", spec, False, shas)
    dve_ops_mod.OPS.append(op)
    dve_ops_mod.CUSTOM_DVE_SPECS["CLIP_MUL_ANT"] = spec
    return op


F32 = mybir.dt.float32
F16 = mybir.dt.float16

H = 384
W = 384
K = 4
N_CORES = 8
N_LOC = 6                  # (B*CH) rows per core
NCO = 8                    # coarse exp samples per 128-row chunk (per k)
NMID = 16                  # mid-grid ratio samples per chunk
NCHUNK = 18                # real chunks per core (6 rows x 3)
NPAD = 20                  # incl 2 dummy chunks -> 5 full ACT groups
NGRP = 5                   # ACT groups (4 chunks each)
SUPERS = ((0, (0, 1)), (1, (2, 3)), (2, (4,)))  # super -> ACT groups
MARGIN_C = 3.0             # coarse grid margin (pixels)
MARGIN_M = 2.0             # mid grid margin
TAU = 1e-3                 # |scale| threshold for the qc-fold fallback

_cache = {}


def _build_nc():
    nc = bacc.Bacc(target_bir_lowering=False)

    # consts: [:, 2g] scale col; [:, 2g+1] bias col
    consts_d = nc.dram_tensor("consts", [128, 2 * NGRP], F32,
                              kind="ExternalInput")
    # per-group ACT input tile: y + qc/scale (or qc where |scale|<tau)
    ybcq_d = nc.dram_tensor("ybcq", [128, NGRP, W], F32, kind="ExternalInput")
    statn_d = nc.dram_tensor("statn", [128, NGRP, 4 * NMID], F16,
                             kind="ExternalInput")
    statd_d = nc.dram_tensor("statd", [128, 4 * NMID], F16, kind="ExternalInput")
    # mid->fine interp: 2 variants (chunk parity in 32-row strip), rows
    # replicated per 32-strip so any strip slice has the right content
    m2_d = nc.dram_tensor("m2pad", [128, 2, 128], F16, kind="ExternalInput")
    out_d = nc.dram_tensor("out", [128, NPAD, W], F16, kind="ExternalOutput")

    clip_op = _ensure_clip_mul_op()

    with TileContext(nc) as tc:
        with (
            tc.tile_pool(name="const", bufs=1) as constp,
            tc.tile_pool(name="e1p", bufs=NGRP) as e1p,
            tc.tile_pool(name="rp", bufs=3) as rp,
            tc.tile_pool(name="ymidp", bufs=3) as ymidp,
            tc.tile_pool(name="youtp", bufs=4) as youtp,
            tc.tile_pool(name="nump", bufs=2, space="PSUM") as nump,
            tc.tile_pool(name="denp", bufs=2, space="PSUM") as denp,
            tc.tile_pool(name="yfp", bufs=2, space="PSUM") as yfp,
        ):
            # dummy activation first: hoists the exp ACT_TABLE_LOAD to t=0
            # so it overlaps the input DMAs
            warm = constp.tile([1, 2], F32)
            nc.vector.memset(warm[:, 0:1], 0.0)
            nc.scalar.activation(warm[:, 1:2], warm[:, 0:1],
                                 mybir.ActivationFunctionType.Exp)

            consts = constp.tile([128, 2 * NGRP], F32)
            nc.gpsimd.dma_start(out=consts[:], in_=consts_d[:])
            ybcq = constp.tile([128, NGRP, W], F32)
            nc.gpsimd.dma_start(out=ybcq[:, 0, :], in_=ybcq_d[:, 0, :])
            nc.gpsimd.dma_start(out=ybcq[:, 1:3, :], in_=ybcq_d[:, 1:3, :])
            nc.gpsimd.dma_start(out=ybcq[:, 3:5, :], in_=ybcq_d[:, 3:5, :])
            statn = constp.tile([128, NGRP, 4 * NMID], F16)
            nc.gpsimd.dma_start(out=statn[:], in_=statn_d[:])
            statd = constp.tile([128, 4 * NMID], F16)
            nc.gpsimd.dma_start(out=statd[:], in_=statd_d[:])
            m2 = constp.tile([128, 2, 128], F16)
            nc.gpsimd.dma_start(out=m2[:], in_=m2_d[:])

            num_s = {}
            den_s = {}
            ymid_s = {}

            def do_contract(s, groups):
                num_s[s] = nump.tile([128, W], F32, name=f"num{s}", tag="num")
                den_s[s] = denp.tile([128, W], F32, name=f"den{s}", tag="den")
                for gl, g in enumerate(groups):
                    e1 = e1p.tile([128, W], F16)
                    nc.scalar.activation(
                        e1[:], ybcq[:, g, :], mybir.ActivationFunctionType.Exp,
                        bias=consts[:, 2 * g + 1:2 * g + 2],
                        scale=consts[:, 2 * g:2 * g + 1],
                    )
                    rows = 64 * gl
                    nc.tensor.matmul(num_s[s][rows:rows + 64, :],
                                     statn[:, g, :], e1[:],
                                     start=True, stop=True)
                    nc.tensor.matmul(den_s[s][rows:rows + 64, :],
                                     statd[:], e1[:],
                                     start=True, stop=True)

            def do_finalize(s, nrows):
                r = rp.tile([128, W], F32)
                nc.vector.reciprocal_approx_fast(
                    out=r[0:nrows, :], in_=den_s[s][0:nrows, :])
                ymid = ymidp.tile([128, W], F16)
                nc.vector._custom_dve(
                    clip_op, out=ymid[0:nrows, :],
                    in0=num_s[s][0:nrows, :], in1=r[0:nrows, :], s0=1.0)
                ymid_s[s] = ymid

            def do_outputs(s):
                for t0 in range(8 * s, min(8 * s + 8, NCHUNK), 2):
                    yf = yfp.tile([128, 2, 512], F32)
                    for i, t in enumerate((t0, t0 + 1)):
                        l = t - 8 * s          # chunk index within super
                        st = l // 2            # 32-row strip in ymid
                        q2 = l % 2             # position within strip
                        nc.tensor.matmul(
                            yf[:, i, 0:W],
                            m2[32 * st:32 * st + 32, q2, :],
                            ymid_s[s][32 * st:32 * st + 32, :],
                            start=True, stop=True,
                            tile_position=(32 * st, 0),
                        )
                    # PSUM->SBUF f16 eviction (DMA cannot read PSUM);
                    # ScalarE takes every 3rd pair, VectorE the rest
                    yout = youtp.tile([128, 2, W], F16)
                    if (t0 // 2) % 3 == 0:
                        nc.scalar.copy(yout[:], yf[:, :, 0:W])
                    else:
                        nc.vector.tensor_copy(out=yout[:], in_=yf[:, :, 0:W])
                    nc.sync.dma_start(
                        out=out_d[:, t0:t0 + 2, :],
                        in_=yout[:],
                    )

            # software-pipelined: contract super s+1 before finalizing s so
            # the in-order PE queue never stalls on the DVE finalize
            do_contract(0, SUPERS[0][1])
            do_contract(1, SUPERS[1][1])
            do_finalize(0, 128)
            do_outputs(0)
            do_contract(2, SUPERS[2][1])
            do_finalize(1, 128)
            do_outputs(1)
            do_finalize(2, 64)
            do_outputs(2)
    nc.finalize()
    return nc


def _interp_weights(cp, fp, order=4):
    """Lagrange interpolation weights from points cp to points fp."""
    M = np.zeros((len(cp), len(fp)))
    for j, f in enumerate(fp):
        i = int(np.searchsorted(cp, f))
        lo = min(max(i - order // 2, 0), len(cp) - order)
        pts = cp[lo:lo + order]
        for a in range(order):
            L = 1.0
            for b in range(order):
                if a != b:
                    L *= (f - pts[b]) / (pts[a] - pts[b])
            M[lo + a, j] = L
    return M


def _host_precompute(params: np.ndarray):
    """Build per-core derived input arrays (float64 host math)."""
    P = np.asarray(params, dtype=np.float64).reshape(48, 28)
    mu_x = P[:, 0:4]
    mu_y = P[:, 4:8]
    wgt = P[:, 8:12]
    S00 = P[:, 12:28][:, 0::4]
    S10 = P[:, 12:28][:, 2::4]
    S11 = P[:, 12:28][:, 3::4]
    A = S00 ** 2
    Bq = 2.0 * S00 * S10
    C = S10 ** 2 + S11 ** 2

    yg = np.linspace(0.0, 1.0, W)

    # chunk-relative coarse/mid grids (identical geometry for every chunk)
    rel_c = np.linspace(-MARGIN_C, 127.0 + MARGIN_C, NCO)
    rel_m = np.linspace(-MARGIN_M, 127.0 + MARGIN_M, NMID)
    M1 = _interp_weights(rel_c, rel_m)                      # [NCO, NMID]
    M2 = _interp_weights(rel_m, np.arange(128.0))           # [NMID, 128]

    m2pad = np.zeros((128, 2, 128), dtype=np.float16)
    for st in range(4):
        for q2 in range(2):
            m2pad[32 * st + 16 * q2:32 * st + 16 * q2 + 16, q2, :] = \
                M2.astype(np.float16)

    # statd: same for every group; rows (q,k,j) -> cols (q,m)
    statd = np.zeros((128, 4 * NMID), dtype=np.float16)
    for q in range(4):
        for k in range(K):
            statd[32 * q + 8 * k:32 * q + 8 * k + 8,
                  NMID * q:NMID * (q + 1)] = M1.astype(np.float16)

    in_maps = []
    for core in range(N_CORES):
        consts = np.zeros((128, 2 * NGRP), dtype=np.float64)
        ybcq = np.zeros((128, NGRP, W), dtype=np.float64)
        statn = np.zeros((128, NGRP, 4 * NMID), dtype=np.float64)
        for t in range(NPAD):
            g, q = t // 4, t % 4
            rows0 = 32 * q
            if t >= NCHUNK:
                # dummy: e = exp(1*0 + 0) = 1; num weight 0, den from statd
                consts[rows0:rows0 + 32, 2 * g] = 1.0
                continue
            n = t // 3
            c0 = 128 * (t % 3)
            ng = core * N_LOC + n
            cp = (c0 + rel_c) / (H - 1)                     # unit coords
            for k in range(K):
                dxc = cp - mu_x[ng, k]
                qa = -0.5 * A[ng, k] * dxc * dxc
                dxB = -0.5 * Bq[ng, k] * dxc
                scale = dxB
                bias = qa - dxB * mu_y[ng, k]
                dy = yg - mu_y[ng, k]
                qc = -0.5 * C[ng, k] * dy * dy              # [W]
                for j in range(NCO):
                    row = rows0 + 8 * k + j
                    if abs(scale[j]) >= TAU:
                        consts[row, 2 * g] = scale[j]
                        consts[row, 2 * g + 1] = bias[j]
                        ybcq[row, g, :] = yg + qc / scale[j]
                    else:
                        # drop the (tiny) dxB*y term: |error| < tau
                        consts[row, 2 * g] = 1.0
                        consts[row, 2 * g + 1] = bias[j]
                        ybcq[row, g, :] = qc
                statn[rows0 + 8 * k:rows0 + 8 * k + 8,
                      g, NMID * q:NMID * (q + 1)] = M1 * wgt[ng, k]
            # overflow guard for f16 exp output: shift this chunk's rows
            # down uniformly (cancels exactly in num/den). bound of
            # scale*in+bias over y in [0,1]: qc <= 0 so bias+relu(scale)
            # still upper-bounds (scale*y + qc + bias)
            rows = slice(rows0, rows0 + 32)
            m = consts[rows, 2 * g + 1] + \
                np.maximum(consts[rows, 2 * g], 0.0)
            shift = max(0.0, m.max() - 9.0)
            if shift > 0.0:
                consts[rows, 2 * g + 1] -= shift
        in_maps.append({
            "consts": consts.astype(np.float32),
            "ybcq": ybcq.astype(np.float32),
            "statn": statn.astype(np.float32).astype(np.float16),
            "statd": statd,
            "m2pad": m2pad,
        })
    return in_maps


def _run(height, width, params, trace=False, **trace_kwargs):
    assert int(height) == H and int(width) == W, (height, width)
    if "nc" not in _cache:
        _cache["nc"] = _build_nc()
    nc = _cache["nc"]
    in_maps = _host_precompute(params)
    res = run_bass_kernel_spmd(
        nc, in_maps, core_ids=list(range(N_CORES)), trace=trace, **trace_kwargs
    )
    full = np.empty((48, H, W), dtype=np.float32)
    for core in range(N_CORES):
        o = res.results[core]["out"].astype(np.float32)  # [128, 20, 384] f16
        full[core * N_LOC:(core + 1) * N_LOC] = \
            o[:, :NCHUNK, :].transpose(1, 0, 2).reshape(N_LOC, H, W)
    return full.reshape(16, 3, H, W), res


def kernel(height, width, params):
    out, _ = _run(height, width, params)
    return out
